# revision 11
# baseline (speedup 1.0000x reference)
"""Trainium2 Bass kernel for LocalSelfAttention (sliding-window attention).

Reference computation (fp32):
  qkv = x @ W_qkv ; q /= 8 ; sliding window of 7 keys (3 each side, zero-padded)
  attn = softmax(q . k_win + pos_bias) ; out = (attn @ v_win) @ W_out

Sharding: data-parallel over B*HW = 128 independent rows -> 16 rows per core.
Each core processes its rows in 8 pairs (512 tokens per pair).

Per-core layout strategy (bf16 matmuls, fp32 PSUM accumulation):
  - x arrives bf16 (host-cast) -> PE-transpose -> xT [D partitions, tokens]
  - qkT = W_qk^T. @ xT  (q,k dims on partitions, tokens free)
  - V   = xT^T. @ W_v   (tokens on partitions, v dims free)
  - scores ST[key, q] = kT_h^T. @ qT_h per (head, key-chunk, row) on query
    stripes; head pairs interleaved for PE row-group concurrency; attention
    runs a 2-deep software pipeline over head pairs so exp/mask latency of
    pair i hides under scores of pair i+1; transposes of the NEXT token
    pair fill the attention tail before the out projection
  - attn_un = exp(ST) * expB   (expB = host-precomputed exp(pos_bias) band
    mask; zero outside the 7-band -> masks everything; one head's multiply
    on DVE, the other on GpSimd)
  - denom = ones[128,64]^T. @ attn_un per head (col-group packed pairs,
    replicated across 64 partitions); merged [128,512] ec-add +
    reciprocal_approx_fast per head-pair
  - avT[dk, q] = V_chunk^T. @ attn_un; one merged [128,512] normalize
    multiply per head-pair during PSUM->SBUF
  - out = avT^T. @ W_out
"""

import numpy as np
import ml_dtypes

import concourse.bass as bass
import concourse.tile as tile
from concourse import bacc, mybir
from concourse.bass_utils import run_bass_kernel_spmd
from concourse.masks import make_identity

# Problem constants (hardcoded per contract)
B, HW, S, D = 2, 64, 256, 512
HEADS, DK, KSIZE, PAD = 8, 64, 7, 3
HDK = HEADS * DK            # 512
QK = 2 * HDK                # 1024 (q and k dims)
N_CORES = 8
ROWS_PER_CORE = (B * HW) // N_CORES   # 16
PAIRS = ROWS_PER_CORE // 2            # 8
PTOK = 2 * S                          # 512 tokens per pair
P = 128
NCH = S // P                          # 2 key chunks per row
STRIPE = 132                          # query stripe width per key chunk (even)
STRIPE_PAD = 256                      # psum slot per (chunk,row) stripe, bank aligned
STRIPE_START = (0, S - STRIPE)        # stripe start per chunk within a row
NPAIR = HEADS // 2                    # 4 head pairs

F32 = mybir.dt.float32
BF16 = mybir.dt.bfloat16
FP8 = mybir.dt.float8e4

_CACHE = {}


S_X = 8.0          # x pre-scale into fp8e4m3 range
S_WQ = 512.0       # W_q pre-scale (q includes /sqrt(dk))
S_WK = 64.0        # W_k pre-scale


def _host_constants(pos_bias, W_qkv, W_out):
    """Host-precomputed tensors: fp8 interleaved qk weights (q pre-scaled),
    bf16 v weights, expB band mask, head-pair-partitioned edge correction."""
    W1 = W_qkv.astype(np.float32).copy()
    W1[:, :HDK] /= np.sqrt(np.float32(DK))
    # qk half in fp8e4m3 (TRN FP8_EXP4 max 240), DoubleRow interleaved:
    # W18[ki, kd, j, m] = Wqk[(2*kd+j)*128 + ki, m] * scale
    Wqk = W1[:, :QK].copy()
    Wqk[:, :HDK] *= S_WQ
    Wqk[:, HDK:] *= S_WK
    W18 = Wqk.reshape(2, 2, P, QK).transpose(2, 0, 1, 3)  # [ki, kd, j, m]
    W18 = np.clip(W18, -240.0, 240.0).astype(ml_dtypes.float8_e4m3)
    W1v = W1[:, QK:].astype(ml_dtypes.bfloat16)   # [512, 512]
    W2 = W_out.astype(np.float32).astype(ml_dtypes.bfloat16)  # [512, 512]

    pb = pos_bias.astype(np.float32)              # [H, S, KSIZE]
    # expB[j, h, c, q'] : key j (within chunk c), query q = STRIPE_START[c] + q'
    # value exp(pos_bias[h, q, w]) with w = (j_global - q) + PAD if in band else 0
    j = np.arange(P)[:, None, None, None]
    h = np.arange(HEADS)[None, :, None, None]
    c = np.arange(NCH)[None, None, :, None]
    qp = np.arange(STRIPE)[None, None, None, :]
    q_glob = np.array(STRIPE_START)[None, None, :, None] + qp
    j_glob = c * P + j
    w = j_glob - q_glob + PAD
    in_band = (w >= 0) & (w < KSIZE)
    w_c = np.clip(w, 0, KSIZE - 1)
    bias_val = pb[h, q_glob, w_c]
    expB = np.where(in_band, np.exp(bias_val), 0.0).astype(np.float32)
    expB = expB.astype(ml_dtypes.bfloat16)        # [128, H, NCH, STRIPE]

    # edge correction: sum over out-of-range window slots of exp(bias).
    # Layout [128, NPAIR, PTOK]: partition p < 64 -> even head of the pair,
    # p >= 64 -> odd head, matching den psum tiles that hold 2 heads.
    q = np.arange(S)[None, :, None]
    w2 = np.arange(KSIZE)[None, None, :]
    oor = ((q + w2 - PAD) < 0) | ((q + w2 - PAD) >= S)
    ec = (np.exp(pb) * oor).sum(-1)               # [H, S]
    ec_pair = np.concatenate([ec, ec], axis=1)    # [H, PTOK]
    ec2 = np.empty((P, NPAIR, PTOK), np.float32)
    for pi in range(NPAIR):
        ec2[:64, pi, :] = ec_pair[2 * pi][None, :]
        ec2[64:, pi, :] = ec_pair[2 * pi + 1][None, :]
    ec2 = ec2.astype(ml_dtypes.bfloat16)
    return (np.ascontiguousarray(W18), np.ascontiguousarray(W1v), W2,
            expB, np.ascontiguousarray(ec2))


def _build_nc():
    nc = bacc.Bacc(None, target_bir_lowering=False)
    x_d = nc.dram_tensor("x", [ROWS_PER_CORE * S, D], BF16, kind="ExternalInput")
    w18_d = nc.dram_tensor("w18", [P, 2, 2, QK], FP8, kind="ExternalInput")
    w1v_d = nc.dram_tensor("w1v", [D, HDK], BF16, kind="ExternalInput")
    w2_d = nc.dram_tensor("w2", [HDK, D], BF16, kind="ExternalInput")
    expb_d = nc.dram_tensor("expb", [P, HEADS, NCH, STRIPE], BF16, kind="ExternalInput")
    ec_d = nc.dram_tensor("ec", [P, NPAIR, PTOK], BF16, kind="ExternalInput")
    out_d = nc.dram_tensor("out", [ROWS_PER_CORE * S, D], F32, kind="ExternalOutput")

    KO = D // P      # 4 K-chunks for projections
    TC = PTOK // P   # 4 token chunks per pair
    QKC = QK // P    # 8 qk output chunks
    HC = HDK // P    # 4 hdk chunks

    with tile.TileContext(nc) as tc:
        with (
            tc.tile_pool(name="const", bufs=1) as const,
            tc.tile_pool(name="io", bufs=2) as io,
            tc.tile_pool(name="work", bufs=2) as work,
            tc.tile_pool(name="attn", bufs=3) as attnp,
            tc.tile_pool(name="ps_proj", bufs=2, space="PSUM") as ps_proj,
            tc.tile_pool(name="ps_st", bufs=2, space="PSUM") as ps_st,
            tc.tile_pool(name="ps_da", bufs=1, space="PSUM") as ps_da,
        ):
            # ---- first x tile ASAP, then constants in order of need ----
            x_tiles = []
            x0 = io.tile([P, TC, D], BF16, tag="x_bf")
            nc.sync.dma_start(
                x0[:], x_d[0:PTOK, :].rearrange("(tc p) d -> p tc d", p=P))
            x_tiles.append(x0)

            w18_sb = const.tile([P, 2, 2, QK], FP8)
            nc.sync.dma_start(w18_sb[:], w18_d[:])
            w1v_sb = const.tile([P, KO, HDK], BF16)
            for ko in range(KO):
                nc.sync.dma_start(
                    w1v_sb[:, ko, :], w1v_d[ko * P:(ko + 1) * P, :])
            ident = const.tile([P, P], BF16)
            make_identity(nc, ident)
            ones_sb = const.tile([P, 64], BF16)
            nc.vector.memset(ones_sb, 1.0)

            w2_sb = const.tile([P, HC, D], BF16)
            nc.sync.dma_start(w2_sb[:], w2_d.rearrange("(hc ki) n -> ki hc n", ki=P))
            expb_sb = const.tile([P, HEADS, NCH, STRIPE], BF16)
            nc.sync.dma_start(expb_sb[:], expb_d[:])
            ec_sb = const.tile([P, NPAIR, PTOK], BF16)
            nc.sync.dma_start(ec_sb[:], ec_d[:])

            def emit_transpose(pr):
                """x (bf16, tokens on partitions) -> xT [D partitions, tokens]"""
                x_bf = x_tiles[pr]
                xT = work.tile([P, KO, PTOK], BF16, tag="xT")
                xT8 = work.tile([P, KO, PTOK], FP8, tag="xT8")
                for tcc in range(TC):
                    tp = ps_proj.tile([P, KO, P], BF16, tag="p512")
                    for ds in range(KO):
                        nc.tensor.transpose(
                            tp[:, ds, :], x_bf[:, tcc, ds * P:(ds + 1) * P],
                            ident)
                    nc.scalar.activation(
                        xT[:, :, tcc * P:(tcc + 1) * P], tp[:],
                        func=mybir.ActivationFunctionType.Copy)
                    nc.vector.tensor_scalar_mul(
                        xT8[:, :, tcc * P:(tcc + 1) * P], tp[:], S_X)
                return xT, xT8

            def emit_qk(xT8):
                qkT = work.tile([P, QKC, PTOK], BF16, tag="qkT")
                for m in range(QKC):
                    pp = ps_proj.tile([P, PTOK], F32, tag="p512")
                    for kd in range(2):
                        nc.tensor.matmul(
                            pp[:],
                            w18_sb[:, kd, :, m * P:(m + 1) * P],
                            xT8[:, 2 * kd:2 * kd + 2, :],
                            start=(kd == 0), stop=(kd == 1),
                            perf_mode=mybir.MatmulPerfMode.DoubleRow,
                        )
                    nc.scalar.activation(
                        qkT[:, m, :], pp[:],
                        func=mybir.ActivationFunctionType.Copy,
                        scale=(1.0 / (S_X * S_WQ) if m < QKC // 2
                               else 1.0 / (S_X * S_WK)))
                return qkT

            xT_cur = None
            qkT_next = None
            for pr in range(PAIRS):
                if pr == 0:
                    xT_cur = emit_transpose(0)
                    qkT_next = emit_qk(xT_cur[1])
                xT, _ = xT_cur
                qkT = qkT_next
                if pr + 1 < PAIRS:
                    xn = io.tile([P, TC, D], BF16, tag="x_bf")
                    nc.sync.dma_start(
                        xn[:],
                        x_d[(pr + 1) * PTOK:(pr + 2) * PTOK, :].rearrange(
                            "(tc p) d -> p tc d", p=P),
                    )
                    x_tiles.append(xn)

                # ---- v projection: V [tokens, hdk] ----
                v_sb = work.tile([P, TC, HDK], BF16, tag="v_sb")
                for tcc in range(TC):
                    pp = ps_proj.tile([P, PTOK], F32, tag="p512")
                    for ko in range(KO):
                        nc.tensor.matmul(
                            pp[:],
                            xT[:, ko, tcc * P:(tcc + 1) * P],
                            w1v_sb[:, ko, :],
                            start=(ko == 0), stop=(ko == KO - 1),
                        )
                    nc.vector.tensor_copy(v_sb[:, tcc, :], pp[:])

                # ---- attention: 4 head pairs, 2-deep software pipeline ----
                avT = attnp.tile([P, HC, PTOK], BF16, tag="avT")
                stage = {}   # pi -> attn_un pair list

                def emit_scores(pi):
                    h0 = 2 * pi
                    st_a = ps_st.tile([P, NCH, 2, STRIPE_PAD], F32, tag="st")
                    st_b = ps_st.tile([P, NCH, 2, STRIPE_PAD], F32, tag="st")
                    sts = [st_a, st_b]
                    for c in range(NCH):
                        for r in range(2):
                            for k in range(2):
                                h = h0 + k
                                mq = h // 2
                                mk = 4 + h // 2
                                sl = slice(64 * k, 64 * k + 64)
                                nc.tensor.matmul(
                                    sts[k][:, c, r, :STRIPE],
                                    qkT[sl, mk,
                                        r * S + c * P:r * S + (c + 1) * P],
                                    qkT[sl, mq,
                                        r * S + STRIPE_START[c]:
                                        r * S + STRIPE_START[c] + STRIPE],
                                    start=True, stop=True,
                                )
                    attn_uns = []
                    for k in range(2):
                        h = h0 + k
                        attn_un = attnp.tile(
                            [P, NCH, 2, STRIPE], BF16, tag=f"attn_un{k}")
                        nc.scalar.activation(
                            attn_un[:], sts[k][:, :, :, :STRIPE],
                            func=mybir.ActivationFunctionType.Exp)
                        nc.vector.tensor_tensor(
                            attn_un[:], attn_un[:],
                            expb_sb[:, h, :, None, :].to_broadcast(
                                (P, NCH, 2, STRIPE)),
                            mybir.AluOpType.mult,
                        )
                        attn_uns.append(attn_un)
                    stage[pi] = attn_uns

                def emit_denav(pi):
                    h0 = 2 * pi
                    attn_uns = stage.pop(pi)
                    den = ps_da.tile([P, PTOK], F32, tag="den")
                    for r in range(2):
                        for c in range(NCH):
                            for k in range(2):
                                sl = slice(64 * k, 64 * k + 64)
                                tpos = None if k == 0 else (0, 64)
                                nc.tensor.matmul(
                                    den[sl, r * S + STRIPE_START[c]:
                                            r * S + STRIPE_START[c] + STRIPE],
                                    ones_sb[:],
                                    attn_uns[k][:, c, r, :],
                                    start=(r == 0 and c == 0),
                                    stop=(r == 1 and c == NCH - 1),
                                    tile_position=tpos,
                                )
                    recip = attnp.tile([P, PTOK], F32, tag="recip")
                    nc.vector.tensor_tensor(
                        recip[:], den[:], ec_sb[:, pi, :], mybir.AluOpType.add)
                    nc.vector.reciprocal_approx_fast(recip[:], recip[:])
                    avp = ps_da.tile([P, PTOK], F32, tag="av")
                    for r in range(2):
                        for c in range(NCH):
                            for k in range(2):
                                h = h0 + k
                                sl = slice(64 * k, 64 * k + 64)
                                tpos = None if k == 0 else (0, 64)
                                nc.tensor.matmul(
                                    avp[sl, r * S + STRIPE_START[c]:
                                            r * S + STRIPE_START[c] + STRIPE],
                                    v_sb[:, 2 * r + c, h * DK:(h + 1) * DK],
                                    attn_uns[k][:, c, r, :],
                                    start=(r == 0 and c == 0),
                                    stop=(r == 1 and c == NCH - 1),
                                    tile_position=tpos,
                                )
                    nc.vector.tensor_tensor(
                        avT[:, pi, :], avp[:], recip[:], mybir.AluOpType.mult)

                emit_scores(0)
                emit_scores(1)
                emit_scores(2)
                emit_denav(0)
                emit_scores(3)
                emit_denav(1)
                emit_denav(2)
                emit_denav(3)

                # next pair's transposes + qk proj fill the attention
                # tail (norm/reciprocal latency) before the out projection
                if pr + 1 < PAIRS:
                    xT_cur = emit_transpose(pr + 1)
                    qkT_next = emit_qk(xT_cur[1])

                # ---- output projection; out DMA split per token chunk ----
                o_sb = io.tile([P, TC, D], F32, tag="o_sb")
                for tcc in range(TC):
                    pp = ps_proj.tile([P, PTOK], F32, tag="p512")
                    for hc in range(HC):
                        nc.tensor.matmul(
                            pp[:],
                            avT[:, hc, tcc * P:(tcc + 1) * P],
                            w2_sb[:, hc, :],
                            start=(hc == 0), stop=(hc == HC - 1),
                        )
                    nc.vector.tensor_copy(o_sb[:, tcc, :], pp[:])
                    nc.sync.dma_start(
                        out_d[pr * PTOK + tcc * P:
                              pr * PTOK + (tcc + 1) * P, :],
                        o_sb[:, tcc, :],
                    )

    nc.compile()
    return nc


def kernel(inputs, pos_bias, W_qkv, W_out):
    x = np.asarray(inputs, np.float32).astype(ml_dtypes.bfloat16)
    W18, W1v, W2, expB, ec = _host_constants(
        np.asarray(pos_bias), np.asarray(W_qkv), np.asarray(W_out))

    if "nc" not in _CACHE:
        _CACHE["nc"] = _build_nc()
    nc = _CACHE["nc"]

    x_flat = x.reshape(B * HW, S, D)
    in_maps = []
    for core in range(N_CORES):
        shard = x_flat[core * ROWS_PER_CORE:(core + 1) * ROWS_PER_CORE]
        in_maps.append({
            "x": np.ascontiguousarray(shard.reshape(ROWS_PER_CORE * S, D)),
            "w18": W18, "w1v": W1v, "w2": W2, "expb": expB, "ec": ec,
        })
    res = run_bass_kernel_spmd(nc, in_maps, core_ids=list(range(N_CORES)))
    out = np.empty((B * HW, S, D), np.float32)
    for core in range(N_CORES):
        out[core * ROWS_PER_CORE:(core + 1) * ROWS_PER_CORE] = (
            res.results[core]["out"].reshape(ROWS_PER_CORE, S, D))
    return out.reshape(B, HW, S, D)
